# revision 13
# baseline (speedup 1.0000x reference)
"""DecodePredictions (RetinaNet-style decode + NMS) for Trainium2, 8 NeuronCores.

Device side (the memory-bound part): stream all 306900x84 prediction rows,
compute per-anchor max class logit (sigmoid is monotonic, so top-k ordering,
and the confidence threshold at 0.5 <=> logit > 0, are preserved).
Host side (tiny, <=1000 rows): sigmoid, top-k 1000, per-candidate argmax class
id, box decode, greedy NMS over 1000 boxes, final gather to 100 detections.

Sharding: 306900 anchors padded to 307200, split contiguously over 8 cores
(38400 rows each). Per core the stream is 10 chunks of [128 partitions x
k_i anchors x 84 channels] with descending k_i so the exposed tail (last
reduce + output DMA) is small.
"""

import numpy as np

import concourse.bacc as bacc
import concourse.mybir as mybir
import concourse.tile as tile

# ---- problem constants (hardcoded; kernel.py must be self-contained) ----
N_ANCHORS = 306900
NUM_CLASSES = 80
ROW = 4 + NUM_CLASSES  # 84
N_CORES = 8
H = 1280
W = 1280
BOX_VAR = np.array([0.1, 0.1, 0.2, 0.2], dtype=np.float32)
TOP_K = 1000
MAX_DET = 100
IOU_THR = 0.5
CONF_THR = 0.5

# ---- sharding layout ----
PER_CORE = 38400          # 307200 / 8 (padded)
N_PAD = PER_CORE * N_CORES
P = 128                   # SBUF partitions
# Anchors per partition per chunk; descending tail so the final
# reduce + out-DMA chain exposed after the last input DMA is short.
DEFAULT_CHUNKS = [33] * 8 + [24, 12]  # sums to 300 (= PER_CORE / P)

_CACHE = {}


def _build_nc(bufs=3, reps=1, do_reduce=True, chunks=None, split_out=0, alt_queue=False):
    """Per-core program: rowwise max over class logits of [PER_CORE, 84].

    reps>1 repeats the whole stream (same data) for steady-state timing.
    """
    if chunks is None:
        chunks = list(DEFAULT_CHUNKS)
    total_k = sum(chunks)
    assert total_k * P == PER_CORE
    nc = bacc.Bacc(
        "TRN2",
        target_bir_lowering=False,
        debug=False,
        enable_asserts=False,
        num_devices=N_CORES,
    )
    pred = nc.dram_tensor(
        "pred", [PER_CORE, ROW], mybir.dt.float32, kind="ExternalInput"
    ).ap()
    out = nc.dram_tensor(
        "maxlogit", [P, total_k], mybir.dt.float32, kind="ExternalOutput"
    ).ap()

    with tile.TileContext(nc, num_cores=N_CORES) as tc:
        with (
            tc.tile_pool(name="in", bufs=bufs) as in_pool,
            tc.tile_pool(name="out", bufs=1) as out_pool,
        ):
            omax = out_pool.tile([P, total_k], mybir.dt.float32)
            if not do_reduce:
                nc.vector.memset(omax[:], 0.0)
            for _ in range(reps):
                row0 = 0
                col0 = 0
                for ci, ki in enumerate(chunks):
                    rows = P * ki
                    t = in_pool.tile([P, ki * ROW], mybir.dt.float32, tag="t")
                    src = pred[row0 : row0 + rows, :].rearrange(
                        "(p k) c -> p (k c)", p=P
                    )
                    eng = nc.scalar if (alt_queue and ci % 2) else nc.sync
                    eng.dma_start(out=t[:], in_=src)
                    if do_reduce:
                        t3 = t[:].rearrange("p (k c) -> p k c", c=ROW)[:, :, 4:ROW]
                        nc.vector.tensor_reduce(
                            out=omax[:, col0 : col0 + ki],
                            in_=t3,
                            axis=mybir.AxisListType.X,
                            op=mybir.AluOpType.max,
                        )
                    row0 += rows
                    col0 += ki
                if split_out:
                    cut = total_k - split_out
                    nc.sync.dma_start(out=out[:, :cut], in_=omax[:, :cut])
                    nc.sync.dma_start(out=out[:, cut:], in_=omax[:, cut:])
                else:
                    nc.sync.dma_start(out=out, in_=omax[:])
    nc.compile()
    return nc


def _get_runner():
    """Build the per-core program once and wrap it in a persistent jitted
    sharded executor (avoids re-tracing/re-compiling on every kernel() call)."""
    if "runner" in _CACHE:
        return _CACHE["runner"]

    import jax
    from jax.sharding import Mesh, NamedSharding, PartitionSpec

    try:
        from jax.experimental.shard_map import shard_map
    except ImportError:  # newer jax
        from jax import shard_map

    from concourse import bass2jax
    from concourse.bass2jax import _bass_exec_p, install_neuronx_cc_hook

    nc = _build_nc()
    install_neuronx_cc_hook()

    partition_name = nc.partition_id_tensor.name if nc.partition_id_tensor else None
    in_names, out_names, out_avals = [], [], []
    for alloc in nc.m.functions[0].allocations:
        if not isinstance(alloc, mybir.MemoryLocationSet):
            continue
        name = alloc.memorylocations[0].name
        if alloc.kind == "ExternalInput":
            if name != partition_name:
                in_names.append(name)
        elif alloc.kind == "ExternalOutput":
            out_names.append(name)
            out_avals.append(
                jax.core.ShapedArray(tuple(alloc.tensor_shape), mybir.dt.np(alloc.dtype))
            )
    assert in_names == ["pred"] and out_names == ["maxlogit"]
    all_names = in_names + out_names
    if partition_name is not None:
        all_names.append(partition_name)

    def _body(*args):
        operands = list(args)
        if partition_name is not None:
            operands.append(bass2jax.partition_id_tensor())
        outs = _bass_exec_p.bind(
            *operands,
            out_avals=tuple(out_avals),
            in_names=tuple(all_names),
            out_names=tuple(out_names),
            lowering_input_output_aliases=(),
            sim_require_finite=True,
            sim_require_nnan=True,
            nc=nc,
        )
        return tuple(outs)

    devices = jax.devices()[:N_CORES]
    mesh = Mesh(np.asarray(devices), ("core",))
    fn = jax.jit(
        shard_map(
            _body,
            mesh=mesh,
            in_specs=(PartitionSpec("core"),) * 2,
            out_specs=(PartitionSpec("core"),),
            check_rep=False,
        ),
        keep_unused=True,
    )
    sharding = NamedSharding(mesh, PartitionSpec("core"))
    total_k = sum(DEFAULT_CHUNKS)
    zeros_out = jax.device_put(
        np.zeros((N_CORES * P, total_k), np.float32), sharding
    )
    runner = (fn, sharding, zeros_out, jax)
    _CACHE["runner"] = runner
    return runner


def _run_device_spmd(padded):
    """Documented-API path: one run_bass_kernel_spmd call (re-jits per call)."""
    from concourse.bass_utils import run_bass_kernel_spmd

    if "nc" not in _CACHE:
        _CACHE["nc"] = _build_nc()
    in_maps = [
        {"pred": padded[c * PER_CORE : (c + 1) * PER_CORE]} for c in range(N_CORES)
    ]
    res = run_bass_kernel_spmd(_CACHE["nc"], in_maps, list(range(N_CORES)))
    return np.stack([res.results[c]["maxlogit"] for c in range(N_CORES)])


def _run_device(pred_flat):
    """pred_flat: [N_ANCHORS, 84] f32 -> maxlogit [N_ANCHORS] f32."""
    padded = np.zeros((N_PAD, ROW), dtype=np.float32)
    padded[:N_ANCHORS] = pred_flat

    per_core = None
    if not _CACHE.get("runner_broken"):
        try:
            fn, sharding, zeros_out, jax = _get_runner()
            pred_dev = jax.device_put(padded, sharding)
            (out_dev,) = fn(pred_dev, zeros_out)
            per_core = np.asarray(out_dev).reshape(N_CORES, P, -1)
        except Exception:
            _CACHE["runner_broken"] = True
    if per_core is None:
        per_core = _run_device_spmd(padded)  # [N_CORES, P, total_k]

    cols = []
    col0 = 0
    for ki in DEFAULT_CHUNKS:
        # [N_CORES, P, ki] -> rows row0 + p*ki + k per core
        cols.append(per_core[:, :, col0 : col0 + ki].reshape(N_CORES, P * ki))
        col0 += ki
    full = np.concatenate(cols, axis=1).reshape(N_PAD)
    return full[:N_ANCHORS]


def _cpu_device():
    if "cpu" not in _CACHE:
        try:
            import jax

            _CACHE["cpu"] = jax.devices("cpu")[0]
        except Exception:
            _CACHE["cpu"] = None
    return _CACHE["cpu"]


def _sigmoid(x):
    """Bitwise-identical to the reference's eager jax.nn.sigmoid on CPU."""
    cpu = _cpu_device()
    if cpu is not None:
        import jax

        with jax.default_device(cpu):
            return np.asarray(jax.nn.sigmoid(jax.numpy.asarray(x)))
    return (1.0 / (1.0 + np.exp(-x.astype(np.float32)))).astype(np.float32)


def _exp(x):
    """Bitwise-identical to the reference's eager jnp.exp on CPU."""
    cpu = _cpu_device()
    if cpu is not None:
        import jax

        with jax.default_device(cpu):
            return np.asarray(jax.numpy.exp(jax.numpy.asarray(x)))
    return np.exp(x)


def _decode_boxes(rows, anch):
    """rows [M,84] f32 raw predictions, anch [M,4] (cx,cy,w,h). Matches reference."""
    bp = rows[:, :4] * BOX_VAR
    ctr = bp[:, :2] * anch[:, 2:] + anch[:, :2]
    wh = _exp(bp[:, 2:]) * anch[:, 2:]
    corners = np.concatenate([ctr - wh / 2.0, ctr + wh / 2.0], axis=-1)
    hi = np.array([W, H, W, H], dtype=np.float32)
    return np.clip(corners, 0.0, hi)


def _iou_one_to_many(box, boxes):
    lt = np.maximum(box[:2], boxes[:, :2])
    rb = np.minimum(box[2:], boxes[:, 2:])
    wh = np.clip(rb - lt, 0.0, None)
    inter = wh[:, 0] * wh[:, 1]
    a1 = (box[2] - box[0]) * (box[3] - box[1])
    a2 = (boxes[:, 2] - boxes[:, 0]) * (boxes[:, 3] - boxes[:, 1])
    return inter / (a1 + a2 - inter + np.float32(1e-8))


def _nms(boxes, scores):
    """Greedy NMS, MAX_DET iterations, matching the reference semantics."""
    s = np.where(scores > np.float32(CONF_THR), scores, -np.inf).astype(np.float32)
    idx_out = np.full(MAX_DET, -1, dtype=np.int64)
    valid_out = np.zeros(MAX_DET, dtype=bool)
    for j in range(MAX_DET):
        i = int(np.argmax(s))
        if s[i] == -np.inf:
            break
        supp = _iou_one_to_many(boxes[i], boxes) > np.float32(IOU_THR)
        s = np.where(supp, -np.inf, s).astype(np.float32)
        s[i] = -np.inf
        idx_out[j] = i
        valid_out[j] = True
    return idx_out, valid_out


def kernel(predictions, anchors):
    predictions = np.asarray(predictions)
    anchors = np.asarray(anchors, dtype=np.float32)
    pred_flat = np.ascontiguousarray(predictions.reshape(-1, ROW), dtype=np.float32)

    maxlogit = _run_device(pred_flat)

    scores = _sigmoid(maxlogit)
    # top-k 1000 by score desc, ties broken by lower index (lax.top_k semantics).
    # k2 >> TOP_K so a tie group straddling rank 1000 cannot extend past the
    # argpartition cut (observed tie groups are <10 wide).
    k2 = 8192
    cand = np.argpartition(-scores, k2)[:k2]
    order = cand[np.lexsort((cand, -scores[cand]))][:TOP_K]
    if scores[order[-1]] <= scores[cand].min():
        # rank-1000 value ties with the partition frontier: the tie group may
        # extend past the cut, so redo with an exact full stable sort
        order = np.lexsort((np.arange(scores.shape[0]), -scores))[:TOP_K]

    probs = scores[order]
    rows = pred_flat[order]
    ids = np.argmax(rows[:, 4:], axis=1).astype(np.int32)
    boxes = _decode_boxes(rows, anchors[order])

    nidx, valid = _nms(boxes, probs)
    gi = np.maximum(nidx, 0)
    vmask = valid[:, None].astype(boxes.dtype)
    det_boxes = (boxes[gi] * vmask).astype(np.float32)
    det_ids = np.where(valid, ids[gi], -1).astype(np.int32)
    det_probs = np.where(valid, probs[gi], 0.0).astype(np.float32)
    return det_boxes, det_ids, det_probs


# revision 14
# speedup vs baseline: 1.2036x; 1.2036x over previous
"""DecodePredictions (RetinaNet-style decode + NMS) for Trainium2, 8 NeuronCores.

Device side (the memory-bound part): stream all 306900x84 prediction rows,
compute per-anchor max class logit (sigmoid is monotonic, so top-k ordering,
and the confidence threshold at 0.5 <=> logit > 0, are preserved).
Host side (tiny, <=1000 rows): sigmoid, top-k 1000, per-candidate argmax class
id, box decode, greedy NMS over 1000 boxes, final gather to 100 detections.

Sharding: 306900 anchors padded to 307200, split contiguously over 8 cores
(38400 rows each). Per core the stream is 10 chunks of [128 partitions x
k_i anchors x 84 channels] with descending k_i so the exposed tail (last
reduce + output DMA) is small.
"""

import numpy as np

import concourse.bacc as bacc
import concourse.mybir as mybir
import concourse.tile as tile

# ---- problem constants (hardcoded; kernel.py must be self-contained) ----
N_ANCHORS = 306900
NUM_CLASSES = 80
ROW = 4 + NUM_CLASSES  # 84
N_CORES = 8
H = 1280
W = 1280
BOX_VAR = np.array([0.1, 0.1, 0.2, 0.2], dtype=np.float32)
TOP_K = 1000
MAX_DET = 100
IOU_THR = 0.5
CONF_THR = 0.5

# ---- sharding layout ----
PER_CORE = 38400          # 307200 / 8 (padded)
N_PAD = PER_CORE * N_CORES
P = 128                   # SBUF partitions
# Anchors per partition per chunk; descending tail so the final
# reduce + out-DMA chain exposed after the last input DMA is short.
DEFAULT_CHUNKS = [33] * 8 + [24, 12]  # sums to 300 (= PER_CORE / P)

_CACHE = {}


def _build_nc(bufs=3, reps=1, do_reduce=True, chunks=None, split_out=12, alt_queue=False):
    """Per-core program: rowwise max over class logits of [PER_CORE, 84].

    reps>1 repeats the whole stream (same data) for steady-state timing.
    """
    if chunks is None:
        chunks = list(DEFAULT_CHUNKS)
    total_k = sum(chunks)
    assert total_k * P == PER_CORE
    nc = bacc.Bacc(
        "TRN2",
        target_bir_lowering=False,
        debug=False,
        enable_asserts=False,
        num_devices=N_CORES,
    )
    pred = nc.dram_tensor(
        "pred", [PER_CORE, ROW], mybir.dt.float32, kind="ExternalInput"
    ).ap()
    out = nc.dram_tensor(
        "maxlogit", [P, total_k], mybir.dt.float32, kind="ExternalOutput"
    ).ap()

    with tile.TileContext(nc, num_cores=N_CORES) as tc:
        with (
            tc.tile_pool(name="in", bufs=bufs) as in_pool,
            tc.tile_pool(name="out", bufs=1) as out_pool,
        ):
            omax = out_pool.tile([P, total_k], mybir.dt.float32)
            if not do_reduce:
                nc.vector.memset(omax[:], 0.0)
            for _ in range(reps):
                row0 = 0
                col0 = 0
                for ci, ki in enumerate(chunks):
                    rows = P * ki
                    t = in_pool.tile([P, ki * ROW], mybir.dt.float32, tag="t")
                    src = pred[row0 : row0 + rows, :].rearrange(
                        "(p k) c -> p (k c)", p=P
                    )
                    eng = nc.scalar if (alt_queue and ci % 2) else nc.sync
                    eng.dma_start(out=t[:], in_=src)
                    if do_reduce:
                        t3 = t[:].rearrange("p (k c) -> p k c", c=ROW)[:, :, 4:ROW]
                        nc.vector.tensor_reduce(
                            out=omax[:, col0 : col0 + ki],
                            in_=t3,
                            axis=mybir.AxisListType.X,
                            op=mybir.AluOpType.max,
                        )
                    row0 += rows
                    col0 += ki
                if split_out:
                    cut = total_k - split_out
                    nc.sync.dma_start(out=out[:, :cut], in_=omax[:, :cut])
                    nc.sync.dma_start(out=out[:, cut:], in_=omax[:, cut:])
                else:
                    nc.sync.dma_start(out=out, in_=omax[:])
    nc.compile()
    return nc


def _get_runner():
    """Build the per-core program once and wrap it in a persistent jitted
    sharded executor (avoids re-tracing/re-compiling on every kernel() call)."""
    if "runner" in _CACHE:
        return _CACHE["runner"]

    import jax
    from jax.sharding import Mesh, NamedSharding, PartitionSpec

    try:
        from jax.experimental.shard_map import shard_map
    except ImportError:  # newer jax
        from jax import shard_map

    from concourse import bass2jax
    from concourse.bass2jax import _bass_exec_p, install_neuronx_cc_hook

    nc = _build_nc()
    install_neuronx_cc_hook()

    partition_name = nc.partition_id_tensor.name if nc.partition_id_tensor else None
    in_names, out_names, out_avals = [], [], []
    for alloc in nc.m.functions[0].allocations:
        if not isinstance(alloc, mybir.MemoryLocationSet):
            continue
        name = alloc.memorylocations[0].name
        if alloc.kind == "ExternalInput":
            if name != partition_name:
                in_names.append(name)
        elif alloc.kind == "ExternalOutput":
            out_names.append(name)
            out_avals.append(
                jax.core.ShapedArray(tuple(alloc.tensor_shape), mybir.dt.np(alloc.dtype))
            )
    assert in_names == ["pred"] and out_names == ["maxlogit"]
    all_names = in_names + out_names
    if partition_name is not None:
        all_names.append(partition_name)

    def _body(*args):
        operands = list(args)
        if partition_name is not None:
            operands.append(bass2jax.partition_id_tensor())
        outs = _bass_exec_p.bind(
            *operands,
            out_avals=tuple(out_avals),
            in_names=tuple(all_names),
            out_names=tuple(out_names),
            lowering_input_output_aliases=(),
            sim_require_finite=True,
            sim_require_nnan=True,
            nc=nc,
        )
        return tuple(outs)

    devices = jax.devices()[:N_CORES]
    mesh = Mesh(np.asarray(devices), ("core",))
    fn = jax.jit(
        shard_map(
            _body,
            mesh=mesh,
            in_specs=(PartitionSpec("core"),) * 2,
            out_specs=(PartitionSpec("core"),),
            check_rep=False,
        ),
        keep_unused=True,
    )
    sharding = NamedSharding(mesh, PartitionSpec("core"))
    total_k = sum(DEFAULT_CHUNKS)
    zeros_out = jax.device_put(
        np.zeros((N_CORES * P, total_k), np.float32), sharding
    )
    runner = (fn, sharding, zeros_out, jax)
    _CACHE["runner"] = runner
    return runner


def _run_device_spmd(padded):
    """Documented-API path: one run_bass_kernel_spmd call (re-jits per call)."""
    from concourse.bass_utils import run_bass_kernel_spmd

    if "nc" not in _CACHE:
        _CACHE["nc"] = _build_nc()
    in_maps = [
        {"pred": padded[c * PER_CORE : (c + 1) * PER_CORE]} for c in range(N_CORES)
    ]
    res = run_bass_kernel_spmd(_CACHE["nc"], in_maps, list(range(N_CORES)))
    return np.stack([res.results[c]["maxlogit"] for c in range(N_CORES)])


def _run_device(pred_flat):
    """pred_flat: [N_ANCHORS, 84] f32 -> maxlogit [N_ANCHORS] f32."""
    padded = np.zeros((N_PAD, ROW), dtype=np.float32)
    padded[:N_ANCHORS] = pred_flat

    per_core = None
    if not _CACHE.get("runner_broken"):
        try:
            fn, sharding, zeros_out, jax = _get_runner()
            pred_dev = jax.device_put(padded, sharding)
            (out_dev,) = fn(pred_dev, zeros_out)
            per_core = np.asarray(out_dev).reshape(N_CORES, P, -1)
        except Exception:
            _CACHE["runner_broken"] = True
    if per_core is None:
        per_core = _run_device_spmd(padded)  # [N_CORES, P, total_k]

    cols = []
    col0 = 0
    for ki in DEFAULT_CHUNKS:
        # [N_CORES, P, ki] -> rows row0 + p*ki + k per core
        cols.append(per_core[:, :, col0 : col0 + ki].reshape(N_CORES, P * ki))
        col0 += ki
    full = np.concatenate(cols, axis=1).reshape(N_PAD)
    return full[:N_ANCHORS]


def _cpu_device():
    if "cpu" not in _CACHE:
        try:
            import jax

            _CACHE["cpu"] = jax.devices("cpu")[0]
        except Exception:
            _CACHE["cpu"] = None
    return _CACHE["cpu"]


def _sigmoid(x):
    """Bitwise-identical to the reference's eager jax.nn.sigmoid on CPU."""
    cpu = _cpu_device()
    if cpu is not None:
        import jax

        with jax.default_device(cpu):
            return np.asarray(jax.nn.sigmoid(jax.numpy.asarray(x)))
    return (1.0 / (1.0 + np.exp(-x.astype(np.float32)))).astype(np.float32)


def _exp(x):
    """Bitwise-identical to the reference's eager jnp.exp on CPU."""
    cpu = _cpu_device()
    if cpu is not None:
        import jax

        with jax.default_device(cpu):
            return np.asarray(jax.numpy.exp(jax.numpy.asarray(x)))
    return np.exp(x)


def _decode_boxes(rows, anch):
    """rows [M,84] f32 raw predictions, anch [M,4] (cx,cy,w,h). Matches reference."""
    bp = rows[:, :4] * BOX_VAR
    ctr = bp[:, :2] * anch[:, 2:] + anch[:, :2]
    wh = _exp(bp[:, 2:]) * anch[:, 2:]
    corners = np.concatenate([ctr - wh / 2.0, ctr + wh / 2.0], axis=-1)
    hi = np.array([W, H, W, H], dtype=np.float32)
    return np.clip(corners, 0.0, hi)


def _iou_one_to_many(box, boxes):
    lt = np.maximum(box[:2], boxes[:, :2])
    rb = np.minimum(box[2:], boxes[:, 2:])
    wh = np.clip(rb - lt, 0.0, None)
    inter = wh[:, 0] * wh[:, 1]
    a1 = (box[2] - box[0]) * (box[3] - box[1])
    a2 = (boxes[:, 2] - boxes[:, 0]) * (boxes[:, 3] - boxes[:, 1])
    return inter / (a1 + a2 - inter + np.float32(1e-8))


def _nms(boxes, scores):
    """Greedy NMS, MAX_DET iterations, matching the reference semantics."""
    s = np.where(scores > np.float32(CONF_THR), scores, -np.inf).astype(np.float32)
    idx_out = np.full(MAX_DET, -1, dtype=np.int64)
    valid_out = np.zeros(MAX_DET, dtype=bool)
    for j in range(MAX_DET):
        i = int(np.argmax(s))
        if s[i] == -np.inf:
            break
        supp = _iou_one_to_many(boxes[i], boxes) > np.float32(IOU_THR)
        s = np.where(supp, -np.inf, s).astype(np.float32)
        s[i] = -np.inf
        idx_out[j] = i
        valid_out[j] = True
    return idx_out, valid_out


def kernel(predictions, anchors):
    predictions = np.asarray(predictions)
    anchors = np.asarray(anchors, dtype=np.float32)
    pred_flat = np.ascontiguousarray(predictions.reshape(-1, ROW), dtype=np.float32)

    maxlogit = _run_device(pred_flat)

    scores = _sigmoid(maxlogit)
    # top-k 1000 by score desc, ties broken by lower index (lax.top_k semantics).
    # k2 >> TOP_K so a tie group straddling rank 1000 cannot extend past the
    # argpartition cut (observed tie groups are <10 wide).
    k2 = 8192
    cand = np.argpartition(-scores, k2)[:k2]
    order = cand[np.lexsort((cand, -scores[cand]))][:TOP_K]
    if scores[order[-1]] <= scores[cand].min():
        # rank-1000 value ties with the partition frontier: the tie group may
        # extend past the cut, so redo with an exact full stable sort
        order = np.lexsort((np.arange(scores.shape[0]), -scores))[:TOP_K]

    probs = scores[order]
    rows = pred_flat[order]
    ids = np.argmax(rows[:, 4:], axis=1).astype(np.int32)
    boxes = _decode_boxes(rows, anchors[order])

    nidx, valid = _nms(boxes, probs)
    gi = np.maximum(nidx, 0)
    vmask = valid[:, None].astype(boxes.dtype)
    det_boxes = (boxes[gi] * vmask).astype(np.float32)
    det_ids = np.where(valid, ids[gi], -1).astype(np.int32)
    det_probs = np.where(valid, probs[gi], 0.0).astype(np.float32)
    return det_boxes, det_ids, det_probs
